# revision 6
# baseline (speedup 1.0000x reference)
"""DTM layer (distance-to-measure) Trainium2 kernel.

Math: for each (batch b, grid point n), with squared distances
d2[m] = ||grid_n - x_{b,m}||^2 and wb = 0.3*M, k = ceil(wb):

    dtm = sum_{i<=k} d2_(i) + (wb - k) * d2_(k)       (order statistics)
        = max_T [ wb*T - sum_m relu(T - d2_m) ]        (concave in T)

so no sort/top-k is needed: find T ~= d2_(k) (k-th smallest), then
evaluate F(T) = sum_m min(d2_m, T) - (M - wb)*T, which is first-order
insensitive to the error in T (dF/dT = wb - c(T) ~= 0 at T*).
Output = sqrt(F / wb).

Because of that insensitivity, T only needs count-level accuracy of
O(+-150) out of 4096, so the threshold search runs on a SUBSAMPLE:
one count pass c(T0) over the first S_CNT=2048 points (scaled x2),
one bracketed regula-falsi update, then the exact final pass over all
M points.  T0 = mu - 0.651*sig from per-row moments (computed via a
K=10 fp32 matmul) centers c(T0) at k; empirically (numpy sim of this
exact pipeline) max rel err ~8e-3 vs the 2e-2 gate.

Device mapping (per NeuronCore, grid axis sharded 8 ways):
  - d2 tiles are produced straight into PSUM by K=12 bf16 matmuls:
    features (gx, gy, g2, 1) x (-2x0, -2x1, 1, |x|^2) with each side
    split hi/lo in bf16 and three cross products stacked along K
    ([hi_g; hi_g; lo_g] . [hi_x; lo_x; hi_x]) -> near-fp32 d2 at the
    same N-cycle PE cost as K=4.
  - One PSUM ring of [128, 1024] fp32 tiles (2 banks) x 4 bufs fills
    all 8 banks and keeps PE generation hidden behind the DVE/ACT
    scans in both passes.
  - count pass: per (gt, b) pair 2 tiles; DVE tensor_scalar(is_le,
    accum) on one, ACT activation(Sign, bias=T, scale=-1, accum) on
    the other.
  - final pass: per pair 4 tiles; DVE min-accum on 2, ACT Relu-accum
    on 2, combined as F = sD - gA + (wb - 2048)*T.
"""

import numpy as np

# ---------------- problem constants (hardcoded per contract) ----------------
B = 4            # batches
M = 4096         # points per batch
N = 10201        # grid points (101 x 101)
NCORES = 8
NPC = 1280       # grid points per core, padded (8*1280 = 10240 >= 10201)
NT = NPC // 128  # 10 grid tiles of 128 rows per core
WB = 0.3 * M     # 1228.8
KK = int(np.ceil(WB))  # 1229
S_CNT = 2048     # subsample size for the count pass (scale = M / S_CNT)
NSC = NT * B     # 40 state columns (gt, b)
# Engine-balanced split of a [128, 2048] PSUM tile: DVE scans [0:XDV]
# at (120+x)/0.96 ns, ACT scans [XDV:2048] at (172+2048-x)/1.2 ns;
# equal at x=920 (~1083 ns each).
XDV = 920

_cache = {}


def _build_nc(reps=1):
    import contextlib
    import concourse.bass as bass
    import concourse.tile as tile
    from concourse import bacc, mybir

    f32 = mybir.dt.float32
    Alu = mybir.AluOpType
    Act = mybir.ActivationFunctionType

    nc = bacc.Bacc("TRN2")
    bf16 = mybir.dt.bfloat16
    gmom = nc.dram_tensor("gmom", [10, 2 * B + NPC], f32, kind="ExternalInput")
    gstk = nc.dram_tensor("gstk", [12, NPC], bf16, kind="ExternalInput")
    xstk = nc.dram_tensor("xstk", [12, B, M], bf16, kind="ExternalInput")
    out_d = nc.dram_tensor("out", [128, NSC], f32, kind="ExternalOutput")

    with tile.TileContext(nc) as tc:
        with tc.tile_pool(name="sing", bufs=1) as sing:
            # ---- inputs to SBUF ----
            gm = sing.tile([10, 2 * B + NPC], f32)
            # K=12 stacked bf16 hi/lo operands: d2 = hi_g.hi_x + hi_g.lo_x
            # + lo_g.hi_x in ONE matmul (same N-cycle cost as K=4)
            gsk = sing.tile([12, NPC], bf16)
            xsk = sing.tile([12, B, M], bf16)
            nc.gpsimd.dma_start(gm[:, :], gmom[:, :])
            nc.gpsimd.dma_start(gsk[:, :], gstk[:, :])
            nc.gpsimd.dma_start(xsk[:, :, :], xstk[:, :, :])

            # ---- state tiles [128, NSC], col = gt*B + b ----
            mu = sing.tile([128, NSC], f32)
            e4 = sing.tile([128, NSC], f32)
            sig = sing.tile([128, NSC], f32)
            hi = sing.tile([128, NSC], f32)
            lo = sing.tile([128, NSC], f32)
            c_lo = sing.tile([128, NSC], f32)
            c_hi = sing.tile([128, NSC], f32)
            T = sing.tile([128, NSC], f32)
            cD = sing.tile([128, NSC], f32)   # DVE count partial
            cA = sing.tile([128, NSC], f32)   # ACT sign-sum partial
            cc = sing.tile([128, NSC], f32)   # combined (scaled) count
            wh = sing.tile([128, NSC], mybir.dt.uint8)
            whn = sing.tile([128, NSC], mybir.dt.uint8)
            t1 = sing.tile([128, NSC], f32)
            t2 = sing.tile([128, NSC], f32)
            t3 = sing.tile([128, NSC], f32)
            sD = sing.tile([128, NSC], f32)   # final DVE sum-min partials
            sD2 = sing.tile([128, NSC], f32)
            gA = sing.tile([128, NSC], f32)   # final ACT relu-sum partials
            gA2 = sing.tile([128, NSC], f32)
            Fv = sing.tile([128, NSC], f32)
            outv = sing.tile([128, NSC], f32)
            # scratch sinks for the elementwise outputs of accum ops
            scrD = sing.tile([128, XDV], f32)
            scrA = sing.tile([128, 2048 - XDV], f32)

            def lhsT(gt):
                return gsk[0:12, gt * 128:(gt + 1) * 128]

            def rhs(b, m0, sz):
                return xsk[0:12, b, m0:m0 + sz]

            # ---- phase 0: moments -> mu, e4 ----
            with tc.tile_pool(name="pmom", bufs=2, space="PSUM") as pmom:
                for gt in range(NT):
                    psm = pmom.tile([128, 2 * B], f32, tag="mom")
                    nc.tensor.matmul(
                        psm[:, :],
                        gm[0:10, 2 * B + gt * 128:2 * B + (gt + 1) * 128],
                        gm[0:10, 0:2 * B],
                        start=True, stop=True,
                    )
                    c0 = gt * B
                    nc.vector.tensor_copy(mu[:, c0:c0 + B], psm[:, 0:B])
                    nc.vector.tensor_copy(e4[:, c0:c0 + B], psm[:, B:2 * B])

            # device-side repetition loop for timing (reps=1: no loop)
            rep_ctx = tc.For_i(0, reps, 1) if reps > 1 else contextlib.nullcontext()
            with rep_ctx:
              # sig = sqrt(max(e4 - mu*mu, eps))
              nc.vector.tensor_mul(t1[:, :], mu[:, :], mu[:, :])
              nc.vector.tensor_sub(t2[:, :], e4[:, :], t1[:, :])
              nc.vector.tensor_scalar_max(t2[:, :], t2[:, :], 1e-12)
              nc.scalar.activation(sig[:, :], t2[:, :], Act.Sqrt)
              # hi = mu + 0.67*sig ; T0 = max(mu - 0.651*sig, 0.05*hi)
              # (z=-0.651 empirically centers c(T0) at k)
              nc.vector.scalar_tensor_tensor(
                  hi[:, :], sig[:, :], 0.67, mu[:, :], op0=Alu.mult, op1=Alu.add)
              nc.vector.scalar_tensor_tensor(
                  T[:, :], sig[:, :], -0.651, mu[:, :], op0=Alu.mult, op1=Alu.add)
              nc.vector.tensor_scalar_mul(t1[:, :], hi[:, :], 0.05)
              nc.vector.tensor_max(T[:, :], T[:, :], t1[:, :])
              nc.vector.memset(lo[:, :], 0.0)
              nc.vector.memset(c_lo[:, :], 0.0)
              nc.vector.memset(c_hi[:, :], float(M))

              with tc.tile_pool(name="pd2", bufs=2, space="PSUM") as pd2:
                  def gen_h(gt, b, h):
                      """4 matmuls producing d2[128 x 2048] in PSUM."""
                      ps = pd2.tile([128, 2048], f32, tag="h")
                      for j in range(4):
                          m0 = h * 2048 + j * 512
                          nc.tensor.matmul(
                              ps[:, j * 512:(j + 1) * 512],
                              lhsT(gt), rhs(b, m0, 512),
                              start=True, stop=True,
                          )
                      return ps

                  def scan_pair(ps, col, dve_op, act_fn, d_acc, a_acc):
                      """Balanced region-split scan: DVE [0:XDV], ACT
                      [XDV:2048], both with accum reductions."""
                      nc.vector.tensor_scalar(
                          scrD[:, :], ps[:, 0:XDV],
                          T[:, col:col + 1], None,
                          op0=dve_op, op1=Alu.add,
                          accum_out=d_acc[:, col:col + 1])
                      nc.scalar.activation(
                          scrA[:, :], ps[:, XDV:2048], act_fn,
                          bias=T[:, col:col + 1], scale=-1.0,
                          accum_out=a_acc[:, col:col + 1])

                  # ---- count pass at T0 over cols [0 : S_CNT] ----
                  for gt in range(NT):
                      for b in range(B):
                          col = gt * B + b
                          ps0 = gen_h(gt, b, 0)
                          scan_pair(ps0, col, Alu.is_le, Act.Sign, cD, cA)

                  # combined scaled count: sub-count over 2048 =
                  # cD + 0.5*cA + (2048-XDV)/2 ; cc = 2*sub-count
                  nc.vector.scalar_tensor_tensor(
                      cc[:, :], cD[:, :], 2.0, cA[:, :],
                      op0=Alu.mult, op1=Alu.add)
                  nc.vector.tensor_scalar(
                      cc[:, :], cc[:, :], float(2048 - XDV), None, op0=Alu.add)
                  # bracket update
                  nc.vector.tensor_scalar(
                      wh[:, :], cc[:, :], float(KK), None, op0=Alu.is_ge)
                  nc.vector.copy_predicated(hi[:, :], wh[:, :], T[:, :])
                  nc.vector.copy_predicated(c_hi[:, :], wh[:, :], cc[:, :])
                  nc.vector.tensor_scalar(
                      whn[:, :], wh[:, :], -1.0, 1.0, op0=Alu.mult, op1=Alu.add)
                  nc.vector.copy_predicated(lo[:, :], whn[:, :], T[:, :])
                  nc.vector.copy_predicated(c_lo[:, :], whn[:, :], cc[:, :])
                  # T = lo + (WB - c_lo) * (hi - lo) / max(c_hi - c_lo, 1)
                  nc.vector.tensor_sub(t1[:, :], hi[:, :], lo[:, :])
                  nc.vector.tensor_sub(t2[:, :], c_hi[:, :], c_lo[:, :])
                  nc.vector.tensor_scalar_max(t2[:, :], t2[:, :], 1.0)
                  nc.vector.reciprocal(t2[:, :], t2[:, :])
                  nc.vector.tensor_scalar(
                      t3[:, :], c_lo[:, :], float(WB), -1.0,
                      op0=Alu.subtract, op1=Alu.mult)
                  nc.vector.tensor_mul(t3[:, :], t3[:, :], t1[:, :])
                  nc.vector.tensor_mul(t3[:, :], t3[:, :], t2[:, :])
                  nc.vector.tensor_add(T[:, :], lo[:, :], t3[:, :])

                  # ---- final F pass over all M points ----
                  for gt in range(NT):
                      for b in range(B):
                          col = gt * B + b
                          ps0 = gen_h(gt, b, 0)
                          scan_pair(ps0, col, Alu.min, Act.Relu, sD, gA)
                          ps1 = gen_h(gt, b, 1)
                          scan_pair(ps1, col, Alu.min, Act.Relu, sD2, gA2)
                  nc.vector.tensor_add(sD[:, :], sD[:, :], sD2[:, :])
                  nc.vector.tensor_add(gA[:, :], gA[:, :], gA2[:, :])

              # sD = sum min(d2, T) over DVE regions (2*XDV elems);
              # gA = sum relu(T - d2) over ACT regions, whose min-sum
              # is (M - 2*XDV)*T - gA.
              # F = sD - gA + (WB - 2*XDV)*T ;  out = sqrt(F / WB)
              nc.vector.tensor_sub(Fv[:, :], sD[:, :], gA[:, :])
              nc.vector.scalar_tensor_tensor(
                  Fv[:, :], T[:, :], float(WB - 2 * XDV), Fv[:, :],
                  op0=Alu.mult, op1=Alu.add)
              nc.vector.tensor_scalar_max(Fv[:, :], Fv[:, :], 0.0)
              nc.scalar.activation(outv[:, :], Fv[:, :], Act.Sqrt, scale=1.0 / WB)
              nc.sync.dma_start(out_d[:, :], outv[:, :])

    nc.finalize()
    return nc


def _host_prep(x, grid):
    """Feature/moment layout prep (O(N + M) host work)."""
    x = np.asarray(x, np.float32)
    grid = np.asarray(grid, np.float32)
    gpad = np.zeros((NCORES * NPC, 2), np.float32)
    gpad[:N] = grid
    gx, gy = gpad[:, 0].astype(np.float64), gpad[:, 1].astype(np.float64)
    g2 = gx * gx + gy * gy
    gfeat = np.stack(
        [gx, gy, g2, np.ones_like(gx), g2 * gx, g2 * gy, g2 * g2,
         gx * gx, gx * gy, gy * gy], 0).astype(np.float32)  # [10, 10240]

    x0 = x[..., 0].astype(np.float64)
    x1 = x[..., 1].astype(np.float64)
    xn2 = x0 * x0 + x1 * x1
    xfeat = np.stack(
        [-2.0 * x0, -2.0 * x1, np.ones_like(x0), xn2], 0).astype(np.float32)

    E = lambda a: a.mean(-1)  # per-batch mean, [B]
    z = np.zeros(B)
    o = np.ones(B)
    # E[d2] coefficients against rows (gx, gy, g2, 1, g2gx, g2gy, g4, gx2, gxgy, gy2)
    c_mu = np.stack([-2 * E(x0), -2 * E(x1), o, E(xn2), z, z, z, z, z, z], 0)
    # E[d2^2] coefficients
    c_e4 = np.stack([
        -4 * E(xn2 * x0), -4 * E(xn2 * x1), 2 * E(xn2), E(xn2 * xn2),
        -4 * E(x0), -4 * E(x1), o, 4 * E(x0 * x0), 8 * E(x0 * x1),
        4 * E(x1 * x1)], 0)
    xmom = np.concatenate([c_mu, c_e4], axis=1).astype(np.float32)  # [10, 2B]

    import ml_dtypes
    bf = ml_dtypes.bfloat16

    def split_hl(v32):
        v = v32.astype(np.float64)
        hi = v.astype(bf)
        lo = (v - hi.astype(np.float64)).astype(bf)
        return hi, lo

    # K=12 stacks: d2 = hi_g.hi_x + hi_g.lo_x + lo_g.hi_x via one matmul
    g_hi, g_lo = split_hl(gfeat[0:4])    # [4, 10240] bf16 each
    x_hi, x_lo = split_hl(xfeat)         # [4, B, M] bf16 each
    gstk = np.concatenate([g_hi, g_hi, g_lo], 0)   # [12, 10240]
    xstk = np.concatenate([x_hi, x_lo, x_hi], 0)   # [12, B, M]
    return gfeat, xmom, gstk, xstk


def _in_maps(x, grid):
    gfeat, xmom, gstk, xstk = _host_prep(x, grid)
    return [
        {
            "gmom": np.ascontiguousarray(np.concatenate(
                [xmom, gfeat[:, c * NPC:(c + 1) * NPC]], axis=1)),
            "gstk": np.ascontiguousarray(gstk[:, c * NPC:(c + 1) * NPC]),
            "xstk": xstk,
        }
        for c in range(NCORES)
    ]


def _get_nc():
    if "nc" not in _cache:
        _cache["nc"] = _build_nc()
    return _cache["nc"]


def kernel(x, grid, _trace=False):
    from concourse.bass_utils import run_bass_kernel_spmd

    in_maps = _in_maps(x, grid)
    nc = _get_nc()
    res = run_bass_kernel_spmd(nc, in_maps, core_ids=list(range(NCORES)),
                               trace=_trace)
    _cache["last_result"] = res
    full = np.zeros((B, NCORES * NPC), np.float32)
    for c in range(NCORES):
        o = res.results[c]["out"].reshape(128, NT, B)
        full[:, c * NPC:(c + 1) * NPC] = o.transpose(2, 1, 0).reshape(B, NPC)
    return full[:, :N]
